# revision 23
# baseline (speedup 1.0000x reference)
"""Trainium2 Bass kernel for a local-attention layer (GQA + RoPE + banded mask).

Full computation (reference semantics, f32):
  q = x@wq, k = x@wk, v = x@wv  (B=2, S=2048, D=2048, Hq=16, Hkv=4, hd=128)
  rope(q), rope(k) interleaved-pair style
  banded causal attention, window=1024, softmax
  out = (probs @ v_rep) @ wo

Sharding: 8 cores = (batch b in {0,1}) x (kv-group g in {0..3}).
Core c handles batch c//4, kv head c%4 and its 4 q heads.  Each core
computes a partial (2048, 2048) output (its heads' contribution through
wo rows); host sums the 4 partials per batch.

Per-core kernel strategy (fp16 matmuls = 1 col/cycle, f32 PSUM):
  - X^T is pre-transposed to fp16 on the HOST and kept resident in SBUF
    (64KB/partition), so no PE transposes are needed for projections.
  - QT/KT/VT = W^T X^T accumulated over 16 k-subtiles; the projections of
    chunk c+1 are software-pipelined INTO the attention pair loop of
    chunk c (PE dispatch is in-order, so filler matmuls must be
    program-order interleaved) to hide softmax-chain latency.
  - RoPE applied in "half-split" form on DVE (4 ops per tile via two
    packed trig tables); host pre-permutes wq/wk columns so this matches
    the reference's interleaved-pair rope.  softmax 1/sqrt(hd) is folded
    into the exp activation's input scale.
  - Scores computed transposed ST[j,i] = KT_tile^T @ QT with per-tile
    active column windows [a0,a1) (banded mask makes boundary tiles
    narrow); exp on ScalarE over the active window only (no max
    subtraction -- scores are provably small).
  - Banded mask applied post-exp as a DVE multiply with one of 8 static
    0/1 fp16 mask tiles (also zeroes the never-computed columns).
  - P@V and the softmax denominator (ones-vector matmul) accumulate on
    PE per j-tile; normalization fused into the PSUM->SBUF copy.
  - attn output computed transposed (d, i) so it feeds o-proj as lhsT
    without transposes.
"""

import os
import numpy as np

B, S, D = 2, 2048, 2048
NH, NKV, HD = 16, 4, 128
WINDOW = 1024
ROPE_THETA = 10000.0
HQ = NH // NKV          # q heads per core = 4
QD = HQ * HD            # 512
NK = D // 128           # 16 contraction chunks
CH = 512                # s-chunk size
NCH = S // CH           # 4 chunks
NSUB = CH // 128        # 4 s-subtiles per chunk
# softmax 1/sqrt(hd) is folded into the exp activation's input scale
EXP_SCALE = 1.0 / float(np.sqrt(np.float32(HD)))

_cache = {}


def _host_prep(wq, wk, wv, wo):
    """Per-core weight slices with rope permutation applied."""
    # de-interleave permutation: dev col j <- ref col (2j if j<64 else 2(j-64)+1)
    perm = np.empty(HD, dtype=np.int64)
    perm[:64] = np.arange(64) * 2
    perm[64:] = np.arange(64) * 2 + 1

    wq_p = wq.reshape(D, NH, HD)[:, :, perm].reshape(D, NH * HD)
    wk_p = wk.reshape(D, NKV, HD)[:, :, perm].reshape(D, NKV * HD)

    inv_freq = 1.0 / (ROPE_THETA ** (np.arange(0, HD, 2, dtype=np.float32) / HD))
    t = np.arange(S, dtype=np.float32)
    ang = np.outer(t, inv_freq)             # (S, 64)
    cosT = np.cos(ang).T.astype(np.float16)  # (64, S)
    sinT = np.sin(ang).T.astype(np.float16)
    # rope halves: y_lo = lo*cos - hi*sin ; y_hi = hi*cos + lo*sin
    # t1 = src*trigA (cos both halves), t2 = swap(src)*trigB ([-sin; sin])
    trigA = np.ascontiguousarray(np.concatenate([cosT, cosT], axis=0))
    trigB = np.ascontiguousarray(np.concatenate([-sinT, sinT], axis=0))

    # 8 static mask tiles [jl, il]: causal k: keep il-jl >= 128k;
    # window k: keep jl >= il-128k
    jl = np.arange(128)[:, None]
    il = np.arange(CH)[None, :]
    masks = np.empty((8, 128, CH), dtype=np.float16)
    for k in range(4):
        masks[k] = (il - jl >= 128 * k)
        masks[4 + k] = (jl >= il - 128 * k)
    masks = np.ascontiguousarray(masks.transpose(1, 0, 2).reshape(128, 8 * CH))

    shards = []
    for c in range(8):
        g = c % 4
        shards.append(dict(
            wq=np.ascontiguousarray(wq_p[:, g * QD:(g + 1) * QD]).astype(np.float16),
            wk=np.ascontiguousarray(wk_p[:, g * HD:(g + 1) * HD]).astype(np.float16),
            wv=np.ascontiguousarray(wv[:, g * HD:(g + 1) * HD]).astype(np.float16),
            wo=np.ascontiguousarray(wo[g * QD:(g + 1) * QD, :]).astype(np.float16),
        ))
    return shards, trigA, trigB, masks


def build_kernel(repeats=1):
    import concourse.bass as bass
    import concourse.mybir as mybir
    import concourse.tile as tile
    from concourse import bacc

    f16 = mybir.dt.float16
    f32 = mybir.dt.float32
    EXP = mybir.ActivationFunctionType.Exp
    MUL = mybir.AluOpType.mult

    nc = bacc.Bacc("TRN2", target_bir_lowering=False, debug=False, num_devices=8)

    x_d = nc.dram_tensor("x16", [D, S], f16, kind="ExternalInput").ap()
    wq_d = nc.dram_tensor("wq", [D, QD], f16, kind="ExternalInput").ap()
    wk_d = nc.dram_tensor("wk", [D, HD], f16, kind="ExternalInput").ap()
    wv_d = nc.dram_tensor("wv", [D, HD], f16, kind="ExternalInput").ap()
    wo_d = nc.dram_tensor("wo", [QD, D], f16, kind="ExternalInput").ap()
    tga_d = nc.dram_tensor("trigA", [128, S], f16, kind="ExternalInput").ap()
    tgb_d = nc.dram_tensor("trigB", [128, S], f16, kind="ExternalInput").ap()
    idn_d = nc.dram_tensor("ident", [128, 128], f16, kind="ExternalInput").ap()
    one_d = nc.dram_tensor("ones", [128, 1], f16, kind="ExternalInput").ap()
    msk_d = nc.dram_tensor("masks", [128, 8 * CH], f16, kind="ExternalInput").ap()
    out_d = nc.dram_tensor("out", [S, D], f32, kind="ExternalOutput").ap()

    with tile.TileContext(nc) as tc:
        with (
            tc.tile_pool(name="persist", bufs=1) as pp,
            tc.tile_pool(name="qtpool", bufs=9) as qtp,
            tc.tile_pool(name="tmp", bufs=2) as tp,
            tc.tile_pool(name="ptpool", bufs=3) as ptp,
            tc.tile_pool(name="atpool", bufs=9) as atp,
            tc.tile_pool(name="outsb", bufs=2) as osp,
            tc.tile_pool(name="small", bufs=2) as smp,
            # PSUM: 8 banks of 2KB.  psS 2x[128,1024]f32 (4 banks: scores /
            # o-proj / vni rotation), psD 1x[128,1024]f32 (pvdn), psP
            # 2x[128,512]f32 (2 banks: projection accumulators).
            tc.tile_pool(name="psS", bufs=2, space="PSUM") as psS,
            tc.tile_pool(name="psD", bufs=1, space="PSUM") as psD,
            tc.tile_pool(name="psP", bufs=2, space="PSUM") as psP,
        ):
            # ---- persistent SBUF tensors -------------------------------
            x_sb = pp.tile([128, NK * S], f16, tag="x16")        # [p,(k,i)]
            wq_sb = pp.tile([128, NK * QD], f16, tag="wq")       # [p,(k,qd)]
            wk_sb = pp.tile([128, NK * HD], f16, tag="wk")
            wv_sb = pp.tile([128, NK * HD], f16, tag="wv")
            wo_sb = pp.tile([128, HQ * D], f16, tag="wo")        # [p,(h,e)]
            tga_sb = pp.tile([128, S], f16, tag="tga")
            tgb_sb = pp.tile([128, S], f16, tag="tgb")
            idn_sb = pp.tile([128, 128], f16, tag="idn")
            one_sb = pp.tile([128, 1], f16, tag="one")
            msk_sb = pp.tile([128, 8 * CH], f16, tag="msk")
            kt_sb = pp.tile([128, S], f16, tag="kt")             # rope'd K^T
            v_sb = pp.tile([128, NK * 128], f16, tag="v")        # [p,(jt,d)]

            xv = x_sb[:].rearrange("p (k i) -> p k i", k=NK)
            wqv = wq_sb[:].rearrange("p (k n) -> p k n", k=NK)
            wkv = wk_sb[:].rearrange("p (k n) -> p k n", k=NK)
            wvv = wv_sb[:].rearrange("p (k n) -> p k n", k=NK)
            vv_ = v_sb[:].rearrange("p (j d) -> p j d", j=NK)
            mkv = msk_sb[:].rearrange("p (m i) -> p m i", m=8)

            # DMA order tuned for time-to-first-matmul: chunk-0 x quarters
            # and the first wq k-quarter go first, bulk weights and later x
            # chunks stream behind them across the three DMA-capable queues
            wq_src = wq_d.rearrange("(k p) n -> p k n", p=128)
            xsrc = x_d.rearrange("(k p) i -> p k i", p=128)
            qs = [nc.gpsimd, nc.scalar, nc.sync]
            nc.sync.dma_start(wqv[:, 0:4, :], wq_src[:, 0:4, :])
            for hh in range(4):     # chunk 0 in quarters, 2 queues
                lo = hh * (CH // 4)
                qs[hh % 2].dma_start(
                    xv[:, :, lo:lo + CH // 4], xsrc[:, :, lo:lo + CH // 4])
            for kq in range(1, 4):
                nc.sync.dma_start(wqv[:, 4 * kq:4 * kq + 4, :],
                                  wq_src[:, 4 * kq:4 * kq + 4, :])
            nc.sync.dma_start(wkv, wk_d.rearrange("(k p) n -> p k n", p=128))
            nc.sync.dma_start(wvv, wv_d.rearrange("(k p) n -> p k n", p=128))
            nc.gpsimd.dma_start(tga_sb[:], tga_d)
            nc.scalar.dma_start(idn_sb[:], idn_d)
            nc.scalar.dma_start(one_sb[:], one_d)
            hx = 0
            for c in range(1, NCH):
                for hh in range(2):
                    lo = c * CH + hh * (CH // 2)
                    qs[hx % 3].dma_start(
                        xv[:, :, lo:lo + CH // 2], xsrc[:, :, lo:lo + CH // 2])
                    hx += 1
            nc.scalar.dma_start(
                wo_sb[:].rearrange("p (h n) -> p h n", h=HQ),
                wo_d.rearrange("(h p) n -> p h n", p=128))
            nc.gpsimd.dma_start(tgb_sb[:], tgb_d)
            nc.scalar.dma_start(msk_sb[:], msk_d)

            def rope(dst, src_ps, c):
                """src_ps (128, CH) psum -> dst (128, CH) sbuf, half-split rope."""
                ca = tga_sb[:, c * CH:(c + 1) * CH]
                cb = tgb_sb[:, c * CH:(c + 1) * CH]
                t1 = tp.tile([128, CH], f32, tag="t1")
                t2 = tp.tile([128, CH], f32, tag="t2")
                nc.vector.tensor_mul(t1[:], src_ps, ca)
                nc.vector.tensor_mul(t2[0:64, :], src_ps[64:128, :], cb[0:64, :])
                nc.vector.tensor_mul(t2[64:128, :], src_ps[0:64, :], cb[64:128, :])
                nc.vector.tensor_add(dst, t1[:], t2[:])

            # qts[(c, h)]: roped Q tiles (qtpool ring double-buffers chunks)
            qts = {}

            def proj_ops(c):
                """Yields thunks for chunk c's projections + rope + V prep.

                Draining these inside chunk c-1's attention loop interleaves
                the PE matmuls into softmax-chain bubbles (PE dispatch is
                in-order, so program order is what matters).
                """
                for h in range(HQ):
                    qp = psP.tile([128, CH], f32, tag="proj", name=f"qp{c}_{h}")
                    for kk in range(NK):
                        def mmq(kk=kk, qp=qp, h=h):
                            nc.tensor.matmul(
                                qp[:], wqv[:, kk, h * HD:(h + 1) * HD],
                                xv[:, kk, c * CH:(c + 1) * CH],
                                start=(kk == 0), stop=(kk == NK - 1))
                        yield mmq
                    def rq(qp=qp, h=h):
                        qr = qtp.tile([128, CH], f16, tag="qt", name=f"qr{c}_{h}")
                        rope(qr[:], qp[:], c)
                        qts[(c, h)] = qr
                    yield rq
                kp = psP.tile([128, CH], f32, tag="proj", name=f"kp{c}")
                for kk in range(NK):
                    def mmk(kk=kk, kp=kp):
                        nc.tensor.matmul(
                            kp[:], wkv[:, kk, :],
                            xv[:, kk, c * CH:(c + 1) * CH],
                            start=(kk == 0), stop=(kk == NK - 1))
                    yield mmk
                def rk(kp=kp):
                    rope(kt_sb[:, c * CH:(c + 1) * CH], kp[:], c)
                yield rk
                vp = psP.tile([128, CH], f32, tag="proj", name=f"vp{c}")
                for kk in range(NK):
                    def mmv(kk=kk, vp=vp):
                        nc.tensor.matmul(
                            vp[:], wvv[:, kk, :],
                            xv[:, kk, c * CH:(c + 1) * CH],
                            start=(kk == 0), stop=(kk == NK - 1))
                    yield mmv
                def vt(vp=vp, c=c):
                    # VT psum -> sbuf -> PE transpose -> natural (s, d) f16
                    vt_sb = tp.tile([128, CH], f16, tag="vt", name=f"vt{c}")
                    nc.scalar.copy(vt_sb[:], vp[:])
                    vni = psS.tile([128, 1024], f16, tag="st", name=f"vni{c}")
                    for g in range(NSUB):
                        nc.tensor.transpose(
                            vni[:, g * 128:(g + 1) * 128],
                            vt_sb[:, g * 128:(g + 1) * 128],
                            idn_sb[:])
                    nc.vector.tensor_copy(v_sb[:, c * CH:(c + 1) * CH],
                                          vni[:, 0:CH])
                yield vt

            def oproj_ops(c, ats):
                for g in range(NSUB):
                    for half in range(2):
                        ob = osp.tile([128, 1024], f32, tag="ob",
                                      name=f"ob{c}_{g}{half}")
                        op_ = psS.tile([128, 1024], f32, tag="st",
                                       name=f"op{c}_{g}{half}")
                        for e2 in range(2):
                            ecol = half * 1024 + e2 * CH
                            for h in range(HQ):
                                def mo(e2=e2, h=h, ecol=ecol, op_=op_):
                                    nc.tensor.matmul(
                                        op_[:, e2 * CH:(e2 + 1) * CH],
                                        ats[h][:, g * 128:(g + 1) * 128],
                                        wo_sb[:, h * D + ecol: h * D + ecol + CH],
                                        start=(h == 0), stop=(h == HQ - 1))
                                yield mo
                        def fin(c=c, g=g, half=half, ob=ob, op_=op_):
                            if half == 0:
                                nc.scalar.copy(ob[:], op_[:])
                            else:
                                nc.vector.tensor_copy(ob[:], op_[:])
                            nc.sync.dma_start(
                                out_d[c * CH + g * 128: c * CH + (g + 1) * 128,
                                      half * 1024:(half + 1) * 1024], ob[:])
                        yield fin

            def chain(*gens):
                for g in gens:
                    if g is not None:
                        yield from g

            def drain(gen, n):
                if gen is None:
                    return
                for _ in range(n):
                    try:
                        next(gen)()
                    except StopIteration:
                        return

            for _r in range(repeats):
                prev_oproj = None
                for c in range(NCH):
                    if c == 0:
                        drain(proj_ops(0), 10 ** 6)   # prologue, no overlap
                    nxt = chain(
                        prev_oproj,
                        proj_ops(c + 1) if c + 1 < NCH else None)

                    # ---------- attention for i-chunk c -----------------
                    jt0 = max(0, 4 * c - 8)
                    jts = list(range(jt0, 4 * c + 4))
                    npair = len(jts) // 2
                    # spread the pipelined proj + prev-o-proj filler thunks
                    # evenly over the pair iterations
                    n_fill = (72 if prev_oproj is not None else 0) + (
                        102 if c + 1 < NCH else 0)
                    per_pair = max(1, -(-n_fill // (npair * HQ)))
                    ats = []
                    for h in range(HQ):
                        pvdn = psD.tile([128, 1024], f32, tag="pvdn",
                                        name=f"pvdn{c}_{h}")
                        pv_ps = pvdn[:, 0:CH]
                        dn_ps = pvdn[0:1, CH:2 * CH]
                        for pi in range(0, len(jts), 2):
                            pair = jts[pi:pi + 2]
                            st = psS.tile([128, 1024], f32, tag="st",
                                          name=f"st{c}_{h}_{pi}")
                            aws = []
                            for q_, jt in enumerate(pair):
                                # banded mask => only columns [a0, a1) of
                                # this j-tile's scores can be unmasked
                                off = 128 * jt - CH * c
                                a0 = max(0, off)                      # causal
                                a1 = min(CH, off + 127 + WINDOW + 1)  # window
                                aws.append((a0, a1))
                                nc.tensor.matmul(
                                    st[:, q_ * CH + a0: q_ * CH + a1],
                                    kt_sb[:, jt * 128:(jt + 1) * 128],
                                    qts[(c, h)][:, a0:a1],
                                    start=True, stop=True)
                            drain(nxt, per_pair)
                            pt = ptp.tile([128, 2 * CH], f16, tag="pt",
                                          name=f"pt{c}_{h}_{pi}")
                            # exp over active windows only
                            (a00, a01), (a10, a11) = aws
                            if a01 == CH and a10 == 0:
                                nc.scalar.activation(
                                    pt[:, a00:CH + a11], st[:, a00:CH + a11],
                                    EXP, scale=EXP_SCALE)
                            else:
                                nc.scalar.activation(
                                    pt[:, a00:a01], st[:, a00:a01],
                                    EXP, scale=EXP_SCALE)
                                nc.scalar.activation(
                                    pt[:, CH + a10:CH + a11],
                                    st[:, CH + a10:CH + a11],
                                    EXP, scale=EXP_SCALE)
                            for q_, jt in enumerate(pair):
                                off = 128 * jt - CH * c
                                a0, a1 = aws[q_]
                                ph = pt[:, q_ * CH + a0:q_ * CH + a1]
                                if off >= 0:
                                    mi = off // 128            # causal mask
                                elif off <= -(CH + 128):
                                    mi = 4 + (off + WINDOW) // 128  # window
                                else:
                                    mi = None
                                if mi is not None:
                                    nc.vector.tensor_tensor(
                                        ph, ph, mkv[:, mi, a0:a1], MUL)
                            for q_, jt in enumerate(pair):
                                a0, a1 = aws[q_]
                                ph = pt[:, q_ * CH + a0:q_ * CH + a1]
                                first = (pi == 0 and q_ == 0)
                                last = (pi + 2 >= len(jts) and q_ == 1)
                                # partial-width accumulation: start=True on
                                # the first matmul clears the whole PSUM
                                # bank, so later tiles' first touch of a
                                # column overwrites pending-zero correctly
                                nc.tensor.matmul(
                                    pv_ps[:, a0:a1], vv_[:, jt, :], ph,
                                    start=first, stop=last)
                                nc.tensor.matmul(
                                    dn_ps[:, a0:a1], one_sb[:, 0:1], ph,
                                    start=first, stop=last)
                        rcp = smp.tile([1, CH], f32, tag="rcp", name=f"rcp{h}")
                        nc.vector.reciprocal(rcp[:], dn_ps)
                        rbc = smp.tile([128, CH], f32, tag="rbc", name=f"rbc{h}")
                        nc.gpsimd.partition_broadcast(rbc[:], rcp[:])
                        at = atp.tile([128, CH], f16, tag="at", name=f"at{h}")
                        nc.vector.tensor_tensor(at[:], pv_ps, rbc[:], MUL)
                        ats.append(at)

                    drain(nxt, 10 ** 6)   # finish any leftover filler work

                    # o-proj for this chunk is deferred: it becomes filler
                    # inside chunk c+1's attention loop (last chunk: now)
                    prev_oproj = oproj_ops(c, ats)
                    if c == NCH - 1:
                        drain(prev_oproj, 10 ** 6)
                        prev_oproj = None
    nc.finalize()
    return nc


def _get_nc():
    if "nc" not in _cache:
        _cache["nc"] = build_kernel()
    return _cache["nc"]


def _build_in_maps(x, wq, wk, wv, wo):
    x = np.asarray(x, dtype=np.float32)
    shards, trigA, trigB, masks = _host_prep(
        np.asarray(wq, np.float32), np.asarray(wk, np.float32),
        np.asarray(wv, np.float32), np.asarray(wo, np.float32))

    ident = np.eye(128, dtype=np.float16)
    ones = np.ones((128, 1), dtype=np.float16)
    xT16 = [np.ascontiguousarray(x[b].T).astype(np.float16) for b in range(B)]

    in_maps = []
    for c in range(8):
        b = c // 4
        m = dict(shards[c])
        m.update(x16=xT16[b], trigA=trigA, trigB=trigB, ident=ident,
                 ones=ones, masks=masks)
        in_maps.append(m)
    return in_maps


def bench_slope(r_lo=1, r_hi=5, iters=24):
    """Builds R-repeat NEFFs and returns per-repeat kernel ns via slope."""
    ts = {}
    for r in (r_lo, r_hi):
        key = f"nc{r}"
        if key not in _cache:
            _cache[key] = build_kernel(repeats=r)
        ts[r] = _bench_nc(_cache[key], iters=iters)
    return (ts[r_hi] - ts[r_lo]) / (r_hi - r_lo), ts


def bench(iters=64, inner=None):
    return _bench_nc(_get_nc(), iters=iters)


def _bench_nc(nc, iters=24):
    """Wall-clock repeated-execution benchmark of a built kernel.

    Re-runs the NEFF with static device-resident inputs so executions can
    pipeline through the axon dispatch; returns estimated ns/iteration.
    Requires kernel() to have been called once (caches in_maps).
    """
    import time as _time
    import jax
    from jax.sharding import Mesh, PartitionSpec, NamedSharding
    from jax.experimental.shard_map import shard_map
    import concourse.mybir as mybir
    from concourse import bass2jax

    in_maps = _cache["in_maps"]
    n_cores = 8

    bass2jax.install_neuronx_cc_hook()
    partition_name = (nc.partition_id_tensor.name
                      if nc.partition_id_tensor else None)
    in_names, out_names, out_avals, zero_outs = [], [], [], []
    for alloc in nc.m.functions[0].allocations:
        if not isinstance(alloc, mybir.MemoryLocationSet):
            continue
        name = alloc.memorylocations[0].name
        if alloc.kind == "ExternalInput":
            if name != partition_name:
                in_names.append(name)
        elif alloc.kind == "ExternalOutput":
            out_names.append(name)
            shape, dtype = tuple(alloc.tensor_shape), mybir.dt.np(alloc.dtype)
            out_avals.append(jax.core.ShapedArray(shape, dtype))
            zero_outs.append(np.zeros((n_cores * shape[0], *shape[1:]), dtype))
    n_params, n_outs = len(in_names), len(out_names)
    all_names = in_names + out_names
    if partition_name is not None:
        all_names = all_names + [partition_name]
    donate = tuple(range(n_params, n_params + n_outs))

    def _body(*args):
        operands = list(args)
        if partition_name is not None:
            operands.append(bass2jax.partition_id_tensor())
        outs = bass2jax._bass_exec_p.bind(
            *operands,
            out_avals=tuple(out_avals),
            in_names=tuple(all_names),
            out_names=tuple(out_names),
            lowering_input_output_aliases=(),
            sim_require_finite=True,
            sim_require_nnan=True,
            nc=nc,
        )
        return tuple(outs)

    mesh = Mesh(np.asarray(jax.devices()[:n_cores]), ("core",))
    in_specs = (PartitionSpec("core"),) * (n_params + n_outs)
    out_specs = (PartitionSpec("core"),) * n_outs
    sharded = jax.jit(
        shard_map(_body, mesh=mesh, in_specs=in_specs,
                  out_specs=out_specs, check_rep=False),
        keep_unused=True)

    shd = NamedSharding(mesh, PartitionSpec("core"))
    dev_in = [
        jax.device_put(
            np.concatenate([np.asarray(in_maps[c][name])
                            for c in range(n_cores)], axis=0), shd)
        for name in in_names
    ]
    dev_zero = [jax.device_put(z, shd) for z in zero_outs]
    outs = sharded(*dev_in, *dev_zero)
    jax.block_until_ready(outs)
    # a couple of untimed iterations to settle caches/clock
    for _ in range(4):
        outs = sharded(*dev_in, *dev_zero)
    jax.block_until_ready(outs)

    t0 = _time.perf_counter()
    for _ in range(iters):
        outs = sharded(*dev_in, *dev_zero)
    jax.block_until_ready(outs)
    dt = _time.perf_counter() - t0
    del outs
    return dt / iters * 1e9


def kernel(x, wq, wk, wv, wo):
    from concourse.bass_utils import run_bass_kernel_spmd

    in_maps = _build_in_maps(x, wq, wk, wv, wo)
    _cache["in_maps"] = in_maps

    nc = _get_nc()
    res = run_bass_kernel_spmd(
        nc, in_maps, core_ids=list(range(8)),
        trace=bool(int(os.environ.get("KERNEL_TRACE", "0"))),
    )
    _cache["last_result"] = res
    parts = [r["out"] for r in res.results]
    out = np.empty((B, S, D), dtype=np.float32)
    for b in range(B):
        out[b] = parts[4 * b] + parts[4 * b + 1] + parts[4 * b + 2] + parts[4 * b + 3]
    return out
